# revision 21
# baseline (speedup 1.0000x reference)
"""Trainium2 Bass kernel for nn_CNN_ODE (CNN encoder + neural ODE + regressor).

Strategy: pure data parallel over 8 NeuronCores (8192 samples/core), parameters
replicated. Per core, activations live feature-on-partition with the two batch
halves stacked into 128 partitions ([128, 4096] tiles).

Encoder: the conv1d is computed in 14 blocks of 3 output positions; each block
is one K=120 matmul against a 5-position input chunk (host stages x with
duplicated chunk boundaries in a DMA-contiguous [128, group, 14*512] layout).
One shared conv lhsT serves every block. SiLU runs natively on the scalar
engine straight out of PSUM; enc1 contracts the 14 chunks into PSUM.

ODE: the integrator is classic RK4 with 3 fixed steps -- this ODE is so smooth
that 3 RK4 steps match the 50-step dopri5 reference to ~2e-7, far below the
fp16 arithmetic noise (~3e-4) and the 2e-2 tolerance. Reformulated in z-space
(z = W1 y, V = W1@W2): each stage z_i is a fresh closed PSUM accumulation
written only by the tensor engine (scaled-V terms plus an identity +w term),
read once by the scalar engine's tanh. S = sum_steps (h/6)(t1+2t2+2t3+t4) is
accumulated via identity-matmul terms; the regressor consumes y0 and S
directly (y_final = y0 + W2 S + T*b2 folded into the regressor weights).
fp16 operands / fp32 accumulation.
"""

import numpy as np

import concourse.bass as bass
import concourse.bacc as bacc
import concourse.mybir as mybir
from concourse.tile import TileContext
from concourse.bass_utils import run_bass_kernel_spmd

F16 = mybir.dt.float16
F32 = mybir.dt.float32
AF = mybir.ActivationFunctionType

N_CORES = 8
B_TOTAL = 65536
SEQ, IN_DIM, N_KER, KSZ = 40, 24, 36, 3
ENC_DIM, HID, REG = 128, 64, 32
ODE_STEPS = 2  # RK4 steps (see module docstring; @2 steps: ~1e-6 vs reference)
NCHUNK = 14    # conv blocks of 3 output positions


def make_consts(inputs, steps=ODE_STEPS):
    """Host-side precompute of all device weight/bias tensors (fp64 math)."""
    f16 = np.float16
    g = {k: np.asarray(v, dtype=np.float64) for k, v in inputs.items() if k != "x"}
    h = float(g["t_span"][1] - g["t_span"][0]) / steps
    W1, b1 = g["ode1_w"], g["ode1_b"]
    W2, b2 = g["ode2_w"], g["ode2_b"]
    V = W1 @ W2
    cvec = W1 @ b2

    c = {}

    def bd(X):
        out = np.zeros((128, 128))
        out[0:64, 0:64] = X
        out[64:128, 64:128] = X
        return out

    # ---- RK4 slots: 0:(h/2)V' 1:hV' 2:(h/6)V' 3:(h/3)V' 4:I 5:(h/6)I 6:(h/3)I
    osl = np.zeros((128, 7, 128))
    osl[:, 0, :] = bd((h / 2 * V).T)
    osl[:, 1, :] = bd((h * V).T)
    osl[:, 2, :] = bd((h / 6 * V).T)
    osl[:, 3, :] = bd((h / 3 * V).T)
    osl[:, 4, :] = np.eye(128)
    osl[:, 5, :] = np.eye(128) * (h / 6)
    osl[:, 6, :] = np.eye(128) * (h / 3)
    c["osl"] = osl.astype(f16)

    beta = np.zeros((64, 4))
    beta[:, 0] = b1
    beta[:, 1] = b1 + h / 2 * cvec
    beta[:, 2] = b1 + h / 2 * cvec
    beta[:, 3] = b1 + h * cvec
    c["beta"] = np.concatenate([beta, beta], axis=0).astype(np.float32)
    gam = (h * cvec)[:, None]
    c["gamma"] = np.concatenate([gam, gam], axis=0).astype(np.float32)
    c["w1t"] = bd(W1.T).astype(f16)

    # ---- conv: one shared lhsT [128,128]; rows r=24*si+ci (si in 0..4,
    # in-pos = 3j-1+si), cols m=36*so+o (out-pos = 3j+so); k = si - so
    cw = g["conv_w"]  # [36, 24, 3]
    cv3 = np.zeros((128, 128))
    for si in range(5):
        for so in range(3):
            k = si - so
            if 0 <= k < 3:
                for ci in range(24):
                    cv3[24 * si + ci, 36 * so : 36 * so + 36] = cw[:, ci, k]
    c["cv3"] = cv3.astype(f16)
    cb3 = np.zeros((128, 1))
    cb3[0:108, 0] = np.tile(g["conv_b"], 3)
    c["conv_bias"] = cb3.astype(np.float32)

    # ---- enc1: [128, 14, 128]: chunk j rows 36*so+o -> out c, flat o*40+(3j+so)
    e1w = g["enc1_w"]  # [128, 1440]
    e1 = np.zeros((128, NCHUNK, 128))
    for j in range(NCHUNK):
        for so in range(3):
            s = 3 * j + so
            if s >= SEQ:
                continue
            for o in range(36):
                e1[36 * so + o, j, :] = e1w[:, o * 40 + s]
    c["enc1_w"] = e1.astype(f16)
    c["enc1_bias"] = g["enc1_b"][:, None].astype(np.float32)  # [128,1]
    c["enc2_w"] = g["enc2_w"].T.astype(f16)  # [128, 64]
    c["enc2_bias"] = g["enc2_b"][:, None].astype(np.float32)  # [64,1]

    # ---- regressor (y_final = y0 + W2 S + steps*h*b2 folded in)
    R1, br1 = g["reg1_w"], g["reg1_b"]
    R2, br2 = g["reg2_w"], g["reg2_b"]
    r1ybd = np.zeros((128, 64))
    r1ybd[0:64, 0:32] = R1.T
    r1ybd[64:128, 32:64] = R1.T
    c["r1y"] = r1ybd.astype(f16)
    # S never materializes: the regressor takes each tanh stage tile directly
    # as a matmul term with weight (h/6)*c_j*(R1 W2), c_j in {1,2,2,1}.
    rw = np.zeros((128, 4, 64))
    for j, cj in enumerate((1.0, 2.0, 2.0, 1.0)):
        blk = (h / 6 * cj * R1 @ W2).T
        rw[0:64, j, 0:32] = blk
        rw[64:128, j, 32:64] = blk
    c["rw"] = rw.astype(f16)
    bias_r = (R1 @ (steps * h * b2) + br1)[:, None]
    c["bias_r"] = np.tile(bias_r, (4, 1)).astype(np.float32)  # [128,1]
    r2bd = np.zeros((128, 4))
    for b in range(4):
        r2bd[32 * b : 32 * b + 32, b] = R2[0]
    c["r2"] = r2bd.astype(f16)  # [128,4] block-diagonal
    c["br2"] = np.full((128, 1), br2[0], np.float32)
    return c


CONST_SPECS = [
    ("osl", [128, 7, 128], F16),
    ("beta", [128, 4], F32),
    ("gamma", [128, 1], F32),
    ("w1t", [128, 128], F16),
    ("cv3", [128, 128], F16),
    ("conv_bias", [128, 1], F32),
    ("enc1_w", [128, NCHUNK, 128], F16),
    ("enc1_bias", [128, 1], F32),
    ("enc2_w", [128, 64], F16),
    ("enc2_bias", [64, 1], F32),
    ("r1y", [128, 64], F16),
    ("rw", [128, 4, 64], F16),
    ("bias_r", [128, 1], F32),
    ("r2", [128, 4], F16),
    ("br2", [128, 1], F32),
]


def _blob_layout():
    """Pack order + column offsets of consts inside the two dtype blobs."""
    off = {F16: 0, F32: 0}
    lay = {}
    for n, sh, dt in CONST_SPECS:
        cols = int(np.prod(sh[1:]))
        lay[n] = (dt, off[dt], cols, sh)
        off[dt] += cols
    return lay, off[F16], off[F32]


def pack_consts(c):
    lay, n16, n32 = _blob_layout()
    b16 = np.zeros((128, n16), np.float16)
    b32 = np.zeros((128, n32), np.float32)
    for n, (dt, off, cols, sh) in lay.items():
        arr = c[n].reshape(sh[0], cols)
        (b16 if dt == F16 else b32)[: sh[0], off : off + cols] = arr
    return b16, b32


def build_nc(bpc, steps=ODE_STEPS, time_reps=1):
    """Build the per-core Bass program (SPMD; identical on all cores)."""
    nc = bacc.Bacc("TRN2", target_bir_lowering=False)
    HB = bpc // 2            # stacked tile width (half-batch)
    NCH = HB // 512          # chunk-columns
    NW = HB // 1024          # ODE waves of 1024 cols
    NG = bpc // 512          # encoder groups

    x_in = nc.dram_tensor("xd", [128, NG, NCHUNK, 512], F16, kind="ExternalInput")
    out_t = nc.dram_tensor("out", [bpc], F32, kind="ExternalOutput")
    lay, n16, n32 = _blob_layout()
    cb16_in = nc.dram_tensor("cb16", [128, n16], F16, kind="ExternalInput")
    cb32_in = nc.dram_tensor("cb32", [128, n32], F32, kind="ExternalInput")

    with TileContext(nc) as tc:
        import contextlib
        es = contextlib.ExitStack()
        with es:
            cpool = es.enter_context(tc.tile_pool(name="consts", bufs=1))
            big = es.enter_context(tc.tile_pool(name="big", bufs=1))

            # const tiles: two packed blobs -> sliced views
            cb16 = cpool.tile([128, n16], F16, tag="cb16", name="cb16")
            cb32 = cpool.tile([128, n32], F32, tag="cb32", name="cb32")
            nc.sync.dma_start(out=cb16[:], in_=cb16_in[:])
            nc.sync.dma_start(out=cb32[:], in_=cb32_in[:])
            ct = {}
            for n, (dt, off, cols, sh) in lay.items():
                v = (cb16 if dt == F16 else cb32)[: sh[0], off : off + cols]
                if len(sh) == 3:
                    v = v.rearrange("p (a b) -> p a b", b=sh[2])
                ct[n] = v

            # persistent state tiles (w = W1 y, f16: it feeds matmuls as rhs)
            w = big.tile([128, HB], F16, tag="w")
            y0 = big.tile([128, HB], F16, tag="y0")
            # one tanh-stage tile per (step, stage): all flow into the regressor
            tS = [[big.tile([128, HB], F16, tag=f"t{s}_{i}", name=f"t{s}_{i}")
                   for i in range(4)] for s in range(steps)]
            pred_sb = big.tile([128, HB // 2], F32, tag="pred")

            for _rep in range(time_reps):

                # ------------- Phase 1: conv + encoder -------------
                def dest_of_group(g):
                    # group g (512 samples) -> (row offset, chunk-col)
                    h_, cc = (0, g) if g < NG // 2 else (1, g - NG // 2)
                    return 64 * h_, cc

                # one shared [128,1024] PSUM pool spans all phases so the
                # scheduler can overlap the encoder tail with the ODE start;
                # eps closes after the encoder and its banks become wp's.
                with tc.tile_pool(name="sb", bufs=3) as epool, \
                     tc.tile_pool(name="ps", bufs=2, space="PSUM") as ps, \
                     tc.tile_pool(name="scr", bufs=4) as opool:
                    AO = mybir.AluOpType
                    with tc.tile_pool(name="ps2", bufs=2, space="PSUM") as eps, \
                         tc.tile_pool(name="cv_ps", bufs=1,
                                      space="PSUM") as cvp:
                        for g in range(NG):
                            ro, cc = dest_of_group(g)
                            ccols = bass.ts(cc, 512)
                            xt = epool.tile([128, NCHUNK, 512], F16, tag="xt")
                            nc.sync.dma_start(out=xt[:], in_=x_in[:, g])
                            h_t = epool.tile([128, NCHUNK, 512], F16, tag="h")
                            for b in range(NCHUNK // 2):
                                if b % 3 == 2:
                                    cp = cvp.tile([128, 1024], F32, tag="cvp")
                                else:
                                    cp = ps.tile([128, 1024], F32, tag="ps")
                                for hf in range(2):
                                    j = 2 * b + hf
                                    nc.tensor.matmul(
                                        cp[:, bass.ts(hf, 512)], ct["cv3"][:],
                                        xt[:, j, :], start=True, stop=True,
                                        skip_group_check=True)
                                nc.scalar.activation(
                                    h_t[:, 2 * b : 2 * b + 2, :].rearrange(
                                        "p a b -> p (a b)"),
                                    cp[:], AF.Silu, bias=ct["conv_bias"][:])
                            ep = eps.tile([128, 512], F32, tag="ep")
                            for j in range(NCHUNK):
                                nc.tensor.matmul(ep[:], ct["enc1_w"][:, j, :],
                                                 h_t[:, j, :], start=(j == 0),
                                                 stop=(j == NCHUNK - 1),
                                                 skip_group_check=True)
                            e1 = epool.tile([128, 512], F16, tag="e1")
                            nc.vector.tensor_scalar(
                                out=e1[:], in0=ep[:],
                                scalar1=ct["enc1_bias"][:], scalar2=0.0,
                                op0=AO.add, op1=AO.max)
                            tp = eps.tile([128, 512], F32, tag="ep")
                            nc.tensor.matmul(tp[0:64, :], ct["enc2_w"][:], e1[:],
                                             start=True, stop=True,
                                             skip_group_check=True)
                            nc.vector.tensor_scalar_add(
                                out=y0[ro : ro + 64, ccols], in0=tp[0:64, :],
                                scalar1=ct["enc2_bias"][0:64])

                        # w0 = W1 @ y0 (block-diagonal over sample halves)
                        for cc in range(NCH):
                            ccols = bass.ts(cc, 512)
                            wp0 = eps.tile([128, 512], F32, tag="ep")
                            nc.tensor.matmul(wp0[:], ct["w1t"][:], y0[:, ccols],
                                             start=True, stop=True,
                                             skip_group_check=True)
                            nc.vector.tensor_copy(out=w[:, ccols], in_=wp0[:])

                    # ------------- Phase 2: RK4 ODE -------------
                    def mm2(pst, sl, rhs, vcol, start, stop):
                        """One term: 2 block-diagonal matmuls (512-col chunks)."""
                        lw = ct["osl"]
                        for ch in range(2):
                            cols = bass.ds(1024 * vcol + 512 * ch, 512)
                            nc.tensor.matmul(pst[:, 512 * ch : 512 * ch + 512],
                                             lw[:, sl, :], rhs[:, cols],
                                             start=start, stop=stop,
                                             skip_group_check=True)

                    with tc.tile_pool(name="wp_ps", bufs=2,
                                      space="PSUM") as wpp:
                        wp_prev = [None] * NW
                        for n in range(steps):
                            t = tS[n]
                            for v in range(NW):
                                vc = bass.ts(v, 1024)
                                # t1: step 0 from w (SBUF); later steps read the
                                # previous step's wp PSUM directly (its bias
                                # beta3 = b1 + h*cvec matches w' = wp + gamma)
                                if n == 0:
                                    nc.scalar.activation(
                                        t[0][:, vc], w[:, vc], AF.Tanh,
                                        bias=ct["beta"][:, 0:1])
                                else:
                                    nc.scalar.activation(
                                        t[0][:, vc], wp_prev[v][:], AF.Tanh,
                                        bias=ct["beta"][:, 3:4])
                                # z2, z3, z4: closed PSUM groups (V-term + I w)
                                for i, (sl, tin) in enumerate(
                                        [(0, t[0]), (0, t[1]), (1, t[2])]):
                                    zb = ps.tile([128, 1024], F32, tag="ps")
                                    mm2(zb, 4, w, v, True, False)
                                    mm2(zb, sl, tin, v, False, True)
                                    nc.scalar.activation(
                                        t[i + 1][:, vc], zb[:], AF.Tanh,
                                        bias=ct["beta"][:, i + 1 : i + 2])
                                if n == steps - 1:
                                    continue  # final w never consumed
                                # wp = w + (h/6)V(t1+2t2+2t3+t4)  (closed group)
                                wp = wpp.tile([128, 1024], F32, tag="wp")
                                for k, (sl, tin) in enumerate(
                                        [(4, w), (2, t[0]), (3, t[1]),
                                         (3, t[2]), (2, t[3])]):
                                    mm2(wp, sl, tin, v, k == 0, k == 4)
                                wp_prev[v] = wp
                                # w' = wp + gamma (off the critical path: the
                                # next step's t1 reads wp directly)
                                nc.vector.tensor_scalar_add(
                                    out=w[:, vc], in0=wp[:],
                                    scalar1=ct["gamma"][:])

                        # ------------- Phase 3: regressor -------------
                        # pred = R2 relu(R1 y0 + sum_{s,j} rw_j t_sj + bias_r)
                        for pr in range(NCH // 2):
                            rp = ps.tile([128, 1024], F32, tag="ps")
                            for idx in range(2):
                                cc = 2 * pr + idx
                                ccols = bass.ts(cc, 512)
                                orow = slice(64 * idx, 64 * idx + 64)
                                tp_ = (0, 64 * idx)
                                nc.tensor.matmul(rp[orow, 0:512], ct["r1y"][:],
                                                 y0[:, ccols], start=True,
                                                 stop=False, tile_position=tp_,
                                                 skip_group_check=True)
                                for s in range(steps):
                                    for j in range(4):
                                        nc.tensor.matmul(
                                            rp[orow, 0:512], ct["rw"][:, j, :],
                                            tS[s][j][:, ccols], start=False,
                                            stop=(s == steps - 1 and j == 3),
                                            tile_position=tp_,
                                            skip_group_check=True)
                            rr = opool.tile([128, 512], F16, tag="rr")
                            nc.vector.tensor_scalar(
                                out=rr[:], in0=rp[:, 0:512],
                                scalar1=ct["bias_r"][:], scalar2=0.0,
                                op0=AO.add, op1=AO.max)
                            pp = wpp.tile([128, 1024], F32, tag="wp")
                            nc.tensor.matmul(pp[0:4, 0:512], ct["r2"][:], rr[:],
                                             start=True, stop=True,
                                             skip_group_check=True)
                            nc.vector.tensor_scalar_add(
                                out=pred_sb[0:4, bass.ts(pr, 512)],
                                in0=pp[0:4, 0:512], scalar1=ct["br2"][0:4])

                    # out DMA: pred_sb row k=(h,par) -> sample mapping
                    pv = pred_sb.rearrange("p (q n) -> p q n", n=512)
                    ov = out_t.rearrange("(h q par n) -> h par q n",
                                         h=2, par=2, n=512)
                    npair = NCH // 2
                    for k, (h_, par) in enumerate(
                            [(0, 0), (1, 0), (0, 1), (1, 1)]):
                        nc.sync.dma_start(out=ov[h_, par],
                                          in_=pv[k : k + 1, 0:npair, :])
    nc.compile()
    return nc


_CACHE = {}


def _get_nc(bpc, steps):
    key = (bpc, steps)
    if key not in _CACHE:
        _CACHE[key] = build_nc(bpc, steps)
    return _CACHE[key]


def make_in_maps(inputs):
    x = np.asarray(inputs["x"])
    B = x.shape[0]
    bpc = B // N_CORES
    ng = bpc // 512
    # position-major staging [44, 24, B]: pad positions -1..42 (index p+1)
    xq = np.zeros((SEQ + 4, IN_DIM, B), np.float16)
    xq[1 : SEQ + 1] = np.ascontiguousarray(
        x.astype(np.float16).transpose(1, 2, 0))
    consts = make_consts(inputs)
    b16, b32 = pack_consts(consts)
    base = {"cb16": b16, "cb32": b32}
    in_maps = []
    for c in range(N_CORES):
        xd = np.zeros((128, ng, NCHUNK, 512), np.float16)
        # chunk j rows 24*si+ci = in position 3j-1+si (xq index 3j+si)
        for j in range(NCHUNK):
            for si in range(5):
                src = xq[3 * j + si, :, c * bpc : (c + 1) * bpc]
                xd[24 * si : 24 * si + 24, :, j, :] = src.reshape(
                    IN_DIM, ng, 512)
        in_maps.append(dict(base, xd=xd))
    return bpc, in_maps


def kernel(**inputs):
    bpc, in_maps = make_in_maps(inputs)
    nc = _get_nc(bpc, ODE_STEPS)
    res = run_bass_kernel_spmd(nc, in_maps, list(range(N_CORES)))
    return np.concatenate([res.results[i]["out"] for i in range(N_CORES)])


# revision 23
# speedup vs baseline: 1.0037x; 1.0037x over previous
"""Trainium2 Bass kernel for nn_CNN_ODE (CNN encoder + neural ODE + regressor).

Strategy: pure data parallel over 8 NeuronCores (8192 samples/core), parameters
replicated. Per core, activations live feature-on-partition with the two batch
halves stacked into 128 partitions ([128, 4096] tiles).

Encoder: the conv1d is computed in 14 blocks of 3 output positions; each block
is one K=120 matmul against a 5-position input chunk (host stages x with
duplicated chunk boundaries in a DMA-contiguous [128, group, 14*512] layout).
One shared conv lhsT serves every block. SiLU runs natively on the scalar
engine straight out of PSUM; enc1 contracts the 14 chunks into PSUM.

ODE: the integrator is classic RK4 with 2 fixed steps -- this ODE is so smooth
that 2 RK4 steps match the 50-step dopri5 reference to ~1e-6, far below the
fp16 arithmetic noise (~3e-4) and the 2e-2 tolerance. Reformulated in z-space
(z = W1 y, V = W1@W2): each stage z_i is a fresh closed PSUM accumulation
written only by the tensor engine (scaled-V term plus an identity +w term),
read once by the scalar engine's tanh -- no cross-engine writes into open
PSUM groups anywhere in the kernel. The next step's t1 reads the w' PSUM
group directly. S never materializes: every tanh stage tile feeds the
regressor as its own matmul term with host-folded weight (h/6)c_j(R1 W2), and
y_final = y0 + W2 S + T*b2 is folded into the regressor weights/bias.
fp16 operands / fp32 accumulation.
"""

import numpy as np

import concourse.bass as bass
import concourse.bacc as bacc
import concourse.mybir as mybir
from concourse.tile import TileContext
from concourse.bass_utils import run_bass_kernel_spmd

F16 = mybir.dt.float16
F32 = mybir.dt.float32
AF = mybir.ActivationFunctionType

N_CORES = 8
B_TOTAL = 65536
SEQ, IN_DIM, N_KER, KSZ = 40, 24, 36, 3
ENC_DIM, HID, REG = 128, 64, 32
ODE_STEPS = 2  # RK4 steps (see module docstring; @2 steps: ~1e-6 vs reference)
NCHUNK = 14    # conv blocks of 3 output positions


def make_consts(inputs, steps=ODE_STEPS):
    """Host-side precompute of all device weight/bias tensors (fp64 math)."""
    f16 = np.float16
    g = {k: np.asarray(v, dtype=np.float64) for k, v in inputs.items() if k != "x"}
    h = float(g["t_span"][1] - g["t_span"][0]) / steps
    W1, b1 = g["ode1_w"], g["ode1_b"]
    W2, b2 = g["ode2_w"], g["ode2_b"]
    V = W1 @ W2
    cvec = W1 @ b2

    c = {}

    def bd(X):
        out = np.zeros((128, 128))
        out[0:64, 0:64] = X
        out[64:128, 64:128] = X
        return out

    # ---- RK4 slots: 0:(h/2)V' 1:hV' 2:(h/6)V' 3:(h/3)V' 4:I 5:(h/6)I 6:(h/3)I
    osl = np.zeros((128, 7, 128))
    osl[:, 0, :] = bd((h / 2 * V).T)
    osl[:, 1, :] = bd((h * V).T)
    osl[:, 2, :] = bd((h / 6 * V).T)
    osl[:, 3, :] = bd((h / 3 * V).T)
    osl[:, 4, :] = np.eye(128)
    osl[:, 5, :] = np.eye(128) * (h / 6)
    osl[:, 6, :] = np.eye(128) * (h / 3)
    c["osl"] = osl.astype(f16)

    beta = np.zeros((64, 4))
    beta[:, 0] = b1
    beta[:, 1] = b1 + h / 2 * cvec
    beta[:, 2] = b1 + h / 2 * cvec
    beta[:, 3] = b1 + h * cvec
    c["beta"] = np.concatenate([beta, beta], axis=0).astype(np.float32)
    gam = (h * cvec)[:, None]
    c["gamma"] = np.concatenate([gam, gam], axis=0).astype(np.float32)
    c["w1t"] = bd(W1.T).astype(f16)

    # ---- conv: one shared lhsT [128,128]; rows r=24*si+ci (si in 0..4,
    # in-pos = 3j-1+si), cols m=36*so+o (out-pos = 3j+so); k = si - so
    cw = g["conv_w"]  # [36, 24, 3]
    cv3 = np.zeros((128, 128))
    for si in range(5):
        for so in range(3):
            k = si - so
            if 0 <= k < 3:
                for ci in range(24):
                    cv3[24 * si + ci, 36 * so : 36 * so + 36] = cw[:, ci, k]
    c["cv3"] = cv3.astype(f16)
    cb3 = np.zeros((128, 1))
    cb3[0:108, 0] = np.tile(g["conv_b"], 3)
    c["conv_bias"] = cb3.astype(np.float32)

    # ---- enc1: [128, 14, 128]: chunk j rows 36*so+o -> out c, flat o*40+(3j+so)
    e1w = g["enc1_w"]  # [128, 1440]
    e1 = np.zeros((128, NCHUNK, 128))
    for j in range(NCHUNK):
        for so in range(3):
            s = 3 * j + so
            if s >= SEQ:
                continue
            for o in range(36):
                e1[36 * so + o, j, :] = e1w[:, o * 40 + s]
    c["enc1_w"] = e1.astype(f16)
    c["enc1_bias"] = g["enc1_b"][:, None].astype(np.float32)  # [128,1]
    c["enc2_w"] = g["enc2_w"].T.astype(f16)  # [128, 64]
    c["enc2_bias"] = g["enc2_b"][:, None].astype(np.float32)  # [64,1]

    # ---- regressor (y_final = y0 + W2 S + steps*h*b2 folded in)
    R1, br1 = g["reg1_w"], g["reg1_b"]
    R2, br2 = g["reg2_w"], g["reg2_b"]
    r1ybd = np.zeros((128, 64))
    r1ybd[0:64, 0:32] = R1.T
    r1ybd[64:128, 32:64] = R1.T
    c["r1y"] = r1ybd.astype(f16)
    # S never materializes: the regressor takes each tanh stage tile directly
    # as a matmul term with weight (h/6)*c_j*(R1 W2), c_j in {1,2,2,1}.
    rw = np.zeros((128, 4, 64))
    for j, cj in enumerate((1.0, 2.0, 2.0, 1.0)):
        blk = (h / 6 * cj * R1 @ W2).T
        rw[0:64, j, 0:32] = blk
        rw[64:128, j, 32:64] = blk
    c["rw"] = rw.astype(f16)
    bias_r = (R1 @ (steps * h * b2) + br1)[:, None]
    c["bias_r"] = np.tile(bias_r, (4, 1)).astype(np.float32)  # [128,1]
    r2bd = np.zeros((128, 4))
    for b in range(4):
        r2bd[32 * b : 32 * b + 32, b] = R2[0]
    c["r2"] = r2bd.astype(f16)  # [128,4] block-diagonal
    c["br2"] = np.full((128, 1), br2[0], np.float32)
    return c


CONST_SPECS = [
    ("osl", [128, 7, 128], F16),
    ("beta", [128, 4], F32),
    ("gamma", [128, 1], F32),
    ("w1t", [128, 128], F16),
    ("cv3", [128, 128], F16),
    ("conv_bias", [128, 1], F32),
    ("enc1_w", [128, NCHUNK, 128], F16),
    ("enc1_bias", [128, 1], F32),
    ("enc2_w", [128, 64], F16),
    ("enc2_bias", [64, 1], F32),
    ("r1y", [128, 64], F16),
    ("rw", [128, 4, 64], F16),
    ("bias_r", [128, 1], F32),
    ("r2", [128, 4], F16),
    ("br2", [128, 1], F32),
]


def _blob_layout():
    """Pack order + column offsets of consts inside the two dtype blobs."""
    off = {F16: 0, F32: 0}
    lay = {}
    for n, sh, dt in CONST_SPECS:
        cols = int(np.prod(sh[1:]))
        lay[n] = (dt, off[dt], cols, sh)
        off[dt] += cols
    return lay, off[F16], off[F32]


def pack_consts(c):
    lay, n16, n32 = _blob_layout()
    b16 = np.zeros((128, n16), np.float16)
    b32 = np.zeros((128, n32), np.float32)
    for n, (dt, off, cols, sh) in lay.items():
        arr = c[n].reshape(sh[0], cols)
        (b16 if dt == F16 else b32)[: sh[0], off : off + cols] = arr
    return b16, b32


def build_nc(bpc, steps=ODE_STEPS, time_reps=1):
    """Build the per-core Bass program (SPMD; identical on all cores)."""
    nc = bacc.Bacc("TRN2", target_bir_lowering=False)
    HB = bpc // 2            # stacked tile width (half-batch)
    NCH = HB // 512          # chunk-columns
    NW = HB // 1024          # ODE waves of 1024 cols
    NG = bpc // 512          # encoder groups

    x_in = nc.dram_tensor("xd", [128, NG, NCHUNK, 512], F16, kind="ExternalInput")
    out_t = nc.dram_tensor("out", [bpc], F32, kind="ExternalOutput")
    lay, n16, n32 = _blob_layout()
    cb16_in = nc.dram_tensor("cb16", [128, n16], F16, kind="ExternalInput")
    cb32_in = nc.dram_tensor("cb32", [128, n32], F32, kind="ExternalInput")

    with TileContext(nc) as tc:
        import contextlib
        es = contextlib.ExitStack()
        with es:
            cpool = es.enter_context(tc.tile_pool(name="consts", bufs=1))
            big = es.enter_context(tc.tile_pool(name="big", bufs=1))

            # const tiles: two packed blobs -> sliced views
            cb16 = cpool.tile([128, n16], F16, tag="cb16", name="cb16")
            cb32 = cpool.tile([128, n32], F32, tag="cb32", name="cb32")
            nc.sync.dma_start(out=cb16[:], in_=cb16_in[:])
            nc.sync.dma_start(out=cb32[:], in_=cb32_in[:])
            ct = {}
            for n, (dt, off, cols, sh) in lay.items():
                v = (cb16 if dt == F16 else cb32)[: sh[0], off : off + cols]
                if len(sh) == 3:
                    v = v.rearrange("p (a b) -> p a b", b=sh[2])
                ct[n] = v

            # persistent state tiles (w = W1 y, f16: it feeds matmuls as rhs)
            w = big.tile([128, HB], F16, tag="w")
            y0 = big.tile([128, HB], F16, tag="y0")
            # one tanh-stage tile per (step, stage): all flow into the regressor
            tS = [[big.tile([128, HB], F16, tag=f"t{s}_{i}", name=f"t{s}_{i}")
                   for i in range(4)] for s in range(steps)]
            pred_sb = big.tile([128, HB // 2], F32, tag="pred")

            for _rep in range(time_reps):

                # ------------- Phase 1: conv + encoder -------------
                def dest_of_group(g):
                    # group g (512 samples) -> (row offset, chunk-col)
                    h_, cc = (0, g) if g < NG // 2 else (1, g - NG // 2)
                    return 64 * h_, cc

                # one shared [128,1024] PSUM pool spans all phases so the
                # scheduler can overlap the encoder tail with the ODE start;
                # eps closes after the encoder and its banks become wp's.
                with tc.tile_pool(name="sb", bufs=3) as epool, \
                     tc.tile_pool(name="ps", bufs=2, space="PSUM") as ps, \
                     tc.tile_pool(name="scr", bufs=4) as opool:
                    AO = mybir.AluOpType
                    with tc.tile_pool(name="ps2", bufs=2, space="PSUM") as eps, \
                         tc.tile_pool(name="cv_ps", bufs=1,
                                      space="PSUM") as cvp:
                        for g in range(NG):
                            ro, cc = dest_of_group(g)
                            ccols = bass.ts(cc, 512)
                            xt = epool.tile([128, NCHUNK, 512], F16, tag="xt")
                            if g == 0:
                                # split so the first conv matmuls start as soon
                                # as the leading chunks land
                                nc.sync.dma_start(out=xt[:, 0:4, :],
                                                  in_=x_in[:, g, 0:4])
                                nc.sync.dma_start(out=xt[:, 4:NCHUNK, :],
                                                  in_=x_in[:, g, 4:NCHUNK])
                            else:
                                nc.sync.dma_start(out=xt[:], in_=x_in[:, g])
                            h_t = epool.tile([128, NCHUNK, 512], F16, tag="h")
                            for b in range(NCHUNK // 2):
                                if b % 3 == 2:
                                    cp = cvp.tile([128, 1024], F32, tag="cvp")
                                else:
                                    cp = ps.tile([128, 1024], F32, tag="ps")
                                for hf in range(2):
                                    j = 2 * b + hf
                                    nc.tensor.matmul(
                                        cp[:, bass.ts(hf, 512)], ct["cv3"][:],
                                        xt[:, j, :], start=True, stop=True,
                                        skip_group_check=True)
                                nc.scalar.activation(
                                    h_t[:, 2 * b : 2 * b + 2, :].rearrange(
                                        "p a b -> p (a b)"),
                                    cp[:], AF.Silu, bias=ct["conv_bias"][:])
                            ep = eps.tile([128, 512], F32, tag="ep")
                            for j in range(NCHUNK):
                                nc.tensor.matmul(ep[:], ct["enc1_w"][:, j, :],
                                                 h_t[:, j, :], start=(j == 0),
                                                 stop=(j == NCHUNK - 1),
                                                 skip_group_check=True)
                            e1 = epool.tile([128, 512], F16, tag="e1")
                            nc.vector.tensor_scalar(
                                out=e1[:], in0=ep[:],
                                scalar1=ct["enc1_bias"][:], scalar2=0.0,
                                op0=AO.add, op1=AO.max)
                            tp = eps.tile([128, 512], F32, tag="ep")
                            nc.tensor.matmul(tp[0:64, :], ct["enc2_w"][:], e1[:],
                                             start=True, stop=True,
                                             skip_group_check=True)
                            nc.vector.tensor_scalar_add(
                                out=y0[ro : ro + 64, ccols], in0=tp[0:64, :],
                                scalar1=ct["enc2_bias"][0:64])

                        # w0 = W1 @ y0 (block-diagonal over sample halves)
                        for cc in range(NCH):
                            ccols = bass.ts(cc, 512)
                            wp0 = eps.tile([128, 512], F32, tag="ep")
                            nc.tensor.matmul(wp0[:], ct["w1t"][:], y0[:, ccols],
                                             start=True, stop=True,
                                             skip_group_check=True)
                            nc.vector.tensor_copy(out=w[:, ccols], in_=wp0[:])

                    # ------------- Phase 2: RK4 ODE -------------
                    def mm2(pst, sl, rhs, vcol, start, stop):
                        """One term: 2 block-diagonal matmuls (512-col chunks)."""
                        lw = ct["osl"]
                        for ch in range(2):
                            cols = bass.ds(1024 * vcol + 512 * ch, 512)
                            nc.tensor.matmul(pst[:, 512 * ch : 512 * ch + 512],
                                             lw[:, sl, :], rhs[:, cols],
                                             start=start, stop=stop,
                                             skip_group_check=True)

                    with tc.tile_pool(name="wp_ps", bufs=2,
                                      space="PSUM") as wpp:
                        wp_prev = [None] * NW
                        for n in range(steps):
                            t = tS[n]
                            for v in range(NW):
                                vc = bass.ts(v, 1024)
                                # t1: step 0 from w (SBUF); later steps read the
                                # previous step's wp PSUM directly (its bias
                                # beta3 = b1 + h*cvec matches w' = wp + gamma)
                                if n == 0:
                                    nc.scalar.activation(
                                        t[0][:, vc], w[:, vc], AF.Tanh,
                                        bias=ct["beta"][:, 0:1])
                                else:
                                    nc.scalar.activation(
                                        t[0][:, vc], wp_prev[v][:], AF.Tanh,
                                        bias=ct["beta"][:, 3:4])
                                # z2, z3, z4: closed PSUM groups (V-term + I w)
                                for i, (sl, tin) in enumerate(
                                        [(0, t[0]), (0, t[1]), (1, t[2])]):
                                    zb = ps.tile([128, 1024], F32, tag="ps")
                                    mm2(zb, 4, w, v, True, False)
                                    mm2(zb, sl, tin, v, False, True)
                                    nc.scalar.activation(
                                        t[i + 1][:, vc], zb[:], AF.Tanh,
                                        bias=ct["beta"][:, i + 1 : i + 2])
                                if n == steps - 1:
                                    continue  # final w never consumed
                                # wp = w + (h/6)V(t1+2t2+2t3+t4)  (closed group)
                                wp = wpp.tile([128, 1024], F32, tag="wp")
                                for k, (sl, tin) in enumerate(
                                        [(4, w), (2, t[0]), (3, t[1]),
                                         (3, t[2]), (2, t[3])]):
                                    mm2(wp, sl, tin, v, k == 0, k == 4)
                                wp_prev[v] = wp
                                # w' = wp + gamma (off the critical path: the
                                # next step's t1 reads wp directly)
                                nc.vector.tensor_scalar_add(
                                    out=w[:, vc], in0=wp[:],
                                    scalar1=ct["gamma"][:])

                        # ------------- Phase 3: regressor -------------
                        # pred = R2 relu(R1 y0 + sum_{s,j} rw_j t_sj + bias_r)
                        for pr in range(NCH // 2):
                            rp = ps.tile([128, 1024], F32, tag="ps")
                            for idx in range(2):
                                cc = 2 * pr + idx
                                ccols = bass.ts(cc, 512)
                                orow = slice(64 * idx, 64 * idx + 64)
                                tp_ = (0, 64 * idx)
                                nc.tensor.matmul(rp[orow, 0:512], ct["r1y"][:],
                                                 y0[:, ccols], start=True,
                                                 stop=False, tile_position=tp_,
                                                 skip_group_check=True)
                                for s in range(steps):
                                    for j in range(4):
                                        nc.tensor.matmul(
                                            rp[orow, 0:512], ct["rw"][:, j, :],
                                            tS[s][j][:, ccols], start=False,
                                            stop=(s == steps - 1 and j == 3),
                                            tile_position=tp_,
                                            skip_group_check=True)
                            rr = opool.tile([128, 512], F16, tag="rr")
                            nc.vector.tensor_scalar(
                                out=rr[:], in0=rp[:, 0:512],
                                scalar1=ct["bias_r"][:], scalar2=0.0,
                                op0=AO.add, op1=AO.max)
                            pp = wpp.tile([128, 1024], F32, tag="wp")
                            nc.tensor.matmul(pp[0:4, 0:512], ct["r2"][:], rr[:],
                                             start=True, stop=True,
                                             skip_group_check=True)
                            nc.vector.tensor_scalar_add(
                                out=pred_sb[0:4, bass.ts(pr, 512)],
                                in0=pp[0:4, 0:512], scalar1=ct["br2"][0:4])

                    # out DMA: pred_sb row k=(h,par) -> sample mapping
                    pv = pred_sb.rearrange("p (q n) -> p q n", n=512)
                    ov = out_t.rearrange("(h q par n) -> h par q n",
                                         h=2, par=2, n=512)
                    npair = NCH // 2
                    for k, (h_, par) in enumerate(
                            [(0, 0), (1, 0), (0, 1), (1, 1)]):
                        nc.sync.dma_start(out=ov[h_, par],
                                          in_=pv[k : k + 1, 0:npair, :])
    nc.compile()
    return nc


_CACHE = {}


def _get_nc(bpc, steps):
    key = (bpc, steps)
    if key not in _CACHE:
        _CACHE[key] = build_nc(bpc, steps)
    return _CACHE[key]


def make_in_maps(inputs):
    x = np.asarray(inputs["x"])
    B = x.shape[0]
    bpc = B // N_CORES
    ng = bpc // 512
    # position-major staging [44, 24, B]: pad positions -1..42 (index p+1)
    xq = np.zeros((SEQ + 4, IN_DIM, B), np.float16)
    xq[1 : SEQ + 1] = np.ascontiguousarray(
        x.astype(np.float16).transpose(1, 2, 0))
    consts = make_consts(inputs)
    b16, b32 = pack_consts(consts)
    base = {"cb16": b16, "cb32": b32}
    in_maps = []
    for c in range(N_CORES):
        xd = np.zeros((128, ng, NCHUNK, 512), np.float16)
        # chunk j rows 24*si+ci = in position 3j-1+si (xq index 3j+si)
        for j in range(NCHUNK):
            for si in range(5):
                src = xq[3 * j + si, :, c * bpc : (c + 1) * bpc]
                xd[24 * si : 24 * si + 24, :, j, :] = src.reshape(
                    IN_DIM, ng, 512)
        in_maps.append(dict(base, xd=xd))
    return bpc, in_maps


def kernel(**inputs):
    bpc, in_maps = make_in_maps(inputs)
    nc = _get_nc(bpc, ODE_STEPS)
    res = run_bass_kernel_spmd(nc, in_maps, list(range(N_CORES)))
    return np.concatenate([res.results[i]["out"] for i in range(N_CORES)])


# revision 26
# speedup vs baseline: 1.0141x; 1.0103x over previous
"""Trainium2 Bass kernel for nn_CNN_ODE (CNN encoder + neural ODE + regressor).

Strategy: pure data parallel over 8 NeuronCores (8192 samples/core), parameters
replicated. Per core, activations live feature-on-partition with the two batch
halves stacked into 128 partitions ([128, 4096] tiles).

Encoder: the conv1d is computed in 14 blocks of 3 output positions; each block
is one K=120 matmul against a 5-position input chunk (host stages x with
duplicated chunk boundaries in a DMA-contiguous [128, group, 14*512] layout).
One shared conv lhsT serves every block. SiLU runs natively on the scalar
engine straight out of PSUM; enc1 contracts the 14 chunks into PSUM.

ODE: the integrator is classic RK4 with 2 fixed steps -- this ODE is so smooth
that 2 RK4 steps match the 50-step dopri5 reference to ~1e-6, far below the
fp16 arithmetic noise (~3e-4) and the 2e-2 tolerance. Reformulated in z-space
(z = W1 y, V = W1@W2): each stage z_i is a fresh closed PSUM accumulation
written only by the tensor engine (scaled-V term plus an identity +w term),
read once by the scalar engine's tanh -- no cross-engine writes into open
PSUM groups anywhere in the kernel. The next step's t1 reads the w' PSUM
group directly. S never materializes: every tanh stage tile feeds the
regressor as its own matmul term with host-folded weight (h/6)c_j(R1 W2), and
y_final = y0 + W2 S + T*b2 is folded into the regressor weights/bias.
fp16 operands / fp32 accumulation.
"""

import numpy as np

import concourse.bass as bass
import concourse.bacc as bacc
import concourse.mybir as mybir
from concourse.tile import TileContext
from concourse.bass_utils import run_bass_kernel_spmd

F16 = mybir.dt.float16
F32 = mybir.dt.float32
AF = mybir.ActivationFunctionType

N_CORES = 8
B_TOTAL = 65536
SEQ, IN_DIM, N_KER, KSZ = 40, 24, 36, 3
ENC_DIM, HID, REG = 128, 64, 32
ODE_STEPS = 2  # RK4 steps (see module docstring; @2 steps: ~1e-6 vs reference)
NCHUNK = 14    # conv blocks of 3 output positions


def make_consts(inputs, steps=ODE_STEPS):
    """Host-side precompute of all device weight/bias tensors (fp64 math)."""
    f16 = np.float16
    g = {k: np.asarray(v, dtype=np.float64) for k, v in inputs.items() if k != "x"}
    h = float(g["t_span"][1] - g["t_span"][0]) / steps
    W1, b1 = g["ode1_w"], g["ode1_b"]
    W2, b2 = g["ode2_w"], g["ode2_b"]
    V = W1 @ W2
    cvec = W1 @ b2

    c = {}

    def bd(X):
        out = np.zeros((128, 128))
        out[0:64, 0:64] = X
        out[64:128, 64:128] = X
        return out

    # ---- RK4 slots: 0:(h/2)V' 1:hV' 2:(h/6)V' 3:(h/3)V' 4:I 5:(h/6)I 6:(h/3)I
    osl = np.zeros((128, 7, 128))
    osl[:, 0, :] = bd((h / 2 * V).T)
    osl[:, 1, :] = bd((h * V).T)
    osl[:, 2, :] = bd((h / 6 * V).T)
    osl[:, 3, :] = bd((h / 3 * V).T)
    osl[:, 4, :] = np.eye(128)
    osl[:, 5, :] = np.eye(128) * (h / 6)
    osl[:, 6, :] = np.eye(128) * (h / 3)
    c["osl"] = osl.astype(f16)

    beta = np.zeros((64, 4))
    beta[:, 0] = b1
    beta[:, 1] = b1 + h / 2 * cvec
    beta[:, 2] = b1 + h / 2 * cvec
    beta[:, 3] = b1 + h * cvec
    c["beta"] = np.concatenate([beta, beta], axis=0).astype(np.float32)
    gam = (h * cvec)[:, None]
    c["gamma"] = np.concatenate([gam, gam], axis=0).astype(np.float32)
    c["w1t"] = bd(W1.T).astype(f16)

    # ---- conv: one shared lhsT [128,128]; rows r=24*si+ci (si in 0..4,
    # in-pos = 3j-1+si), cols m=36*so+o (out-pos = 3j+so); k = si - so
    cw = g["conv_w"]  # [36, 24, 3]
    cv3 = np.zeros((128, 128))
    for si in range(5):
        for so in range(3):
            k = si - so
            if 0 <= k < 3:
                for ci in range(24):
                    cv3[24 * si + ci, 36 * so : 36 * so + 36] = cw[:, ci, k]
    c["cv3"] = cv3.astype(f16)
    cb3 = np.zeros((128, 1))
    cb3[0:108, 0] = np.tile(g["conv_b"], 3)
    c["conv_bias"] = cb3.astype(np.float32)

    # ---- enc1: [128, 14, 128]: chunk j rows 36*so+o -> out c, flat o*40+(3j+so)
    e1w = g["enc1_w"]  # [128, 1440]
    e1 = np.zeros((128, NCHUNK, 128))
    for j in range(NCHUNK):
        for so in range(3):
            s = 3 * j + so
            if s >= SEQ:
                continue
            for o in range(36):
                e1[36 * so + o, j, :] = e1w[:, o * 40 + s]
    c["enc1_w"] = e1.astype(f16)
    c["enc1_bias"] = g["enc1_b"][:, None].astype(np.float32)  # [128,1]
    c["enc2_w"] = g["enc2_w"].T.astype(f16)  # [128, 64]
    c["enc2_bias"] = g["enc2_b"][:, None].astype(np.float32)  # [64,1]

    # ---- regressor (y_final = y0 + W2 S + steps*h*b2 folded in)
    R1, br1 = g["reg1_w"], g["reg1_b"]
    R2, br2 = g["reg2_w"], g["reg2_b"]
    r1ybd = np.zeros((128, 64))
    r1ybd[0:64, 0:32] = R1.T
    r1ybd[64:128, 32:64] = R1.T
    c["r1y"] = r1ybd.astype(f16)
    # S never materializes: the regressor takes each tanh stage tile directly
    # as a matmul term with weight (h/6)*c_j*(R1 W2), c_j in {1,2,2,1}.
    rw = np.zeros((128, 4, 64))
    for j, cj in enumerate((1.0, 2.0, 2.0, 1.0)):
        blk = (h / 6 * cj * R1 @ W2).T
        rw[0:64, j, 0:32] = blk
        rw[64:128, j, 32:64] = blk
    c["rw"] = rw.astype(f16)
    bias_r = (R1 @ (steps * h * b2) + br1)[:, None]
    c["bias_r"] = np.tile(bias_r, (4, 1)).astype(np.float32)  # [128,1]
    r2bd = np.zeros((128, 4))
    for b in range(4):
        r2bd[32 * b : 32 * b + 32, b] = R2[0]
    c["r2"] = r2bd.astype(f16)  # [128,4] block-diagonal
    c["br2"] = np.full((128, 1), br2[0], np.float32)
    return c


CONST_SPECS = [
    ("osl", [128, 7, 128], F16),
    ("beta", [128, 4], F32),
    ("gamma", [128, 1], F32),
    ("w1t", [128, 128], F16),
    ("cv3", [128, 128], F16),
    ("conv_bias", [128, 1], F32),
    ("enc1_w", [128, NCHUNK, 128], F16),
    ("enc1_bias", [128, 1], F32),
    ("enc2_w", [128, 64], F16),
    ("enc2_bias", [64, 1], F32),
    ("r1y", [128, 64], F16),
    ("rw", [128, 4, 64], F16),
    ("bias_r", [128, 1], F32),
    ("r2", [128, 4], F16),
    ("br2", [128, 1], F32),
]


def _blob_layout():
    """Pack order + column offsets of consts inside the two dtype blobs."""
    off = {F16: 0, F32: 0}
    lay = {}
    for n, sh, dt in CONST_SPECS:
        cols = int(np.prod(sh[1:]))
        lay[n] = (dt, off[dt], cols, sh)
        off[dt] += cols
    return lay, off[F16], off[F32]


def pack_consts(c):
    lay, n16, n32 = _blob_layout()
    b16 = np.zeros((128, n16), np.float16)
    b32 = np.zeros((128, n32), np.float32)
    for n, (dt, off, cols, sh) in lay.items():
        arr = c[n].reshape(sh[0], cols)
        (b16 if dt == F16 else b32)[: sh[0], off : off + cols] = arr
    return b16, b32


def build_nc(bpc, steps=ODE_STEPS, time_reps=1):
    """Build the per-core Bass program (SPMD; identical on all cores)."""
    nc = bacc.Bacc("TRN2", target_bir_lowering=False)
    HB = bpc // 2            # stacked tile width (half-batch)
    NCH = HB // 512          # chunk-columns
    NW = HB // 1024          # ODE waves of 1024 cols
    NG = bpc // 512          # encoder groups

    x_in = nc.dram_tensor("xd", [128, NG, NCHUNK, 512], F16, kind="ExternalInput")
    out_t = nc.dram_tensor("out", [bpc], F32, kind="ExternalOutput")
    lay, n16, n32 = _blob_layout()
    cb16_in = nc.dram_tensor("cb16", [128, n16], F16, kind="ExternalInput")
    cb32_in = nc.dram_tensor("cb32", [128, n32], F32, kind="ExternalInput")

    with TileContext(nc) as tc:
        import contextlib
        es = contextlib.ExitStack()
        with es:
            cpool = es.enter_context(tc.tile_pool(name="consts", bufs=1))
            big = es.enter_context(tc.tile_pool(name="big", bufs=1))

            # const tiles: two packed blobs -> sliced views
            cb16 = cpool.tile([128, n16], F16, tag="cb16", name="cb16")
            cb32 = cpool.tile([128, n32], F32, tag="cb32", name="cb32")
            # conv consts land first so the first conv matmuls aren't stalled
            # behind the full const blob
            o16, c16 = lay["cv3"][1], lay["cv3"][2]
            o32, c32 = lay["conv_bias"][1], lay["conv_bias"][2]
            nc.sync.dma_start(out=cb16[:, o16 : o16 + c16],
                              in_=cb16_in[:, o16 : o16 + c16])
            nc.sync.dma_start(out=cb32[:, o32 : o32 + c32],
                              in_=cb32_in[:, o32 : o32 + c32])
            nc.sync.dma_start(out=cb16[:, 0:o16], in_=cb16_in[:, 0:o16])
            nc.sync.dma_start(out=cb16[:, o16 + c16 :],
                              in_=cb16_in[:, o16 + c16 :])
            nc.sync.dma_start(out=cb32[:, 0:o32], in_=cb32_in[:, 0:o32])
            nc.sync.dma_start(out=cb32[:, o32 + c32 :],
                              in_=cb32_in[:, o32 + c32 :])
            ct = {}
            for n, (dt, off, cols, sh) in lay.items():
                v = (cb16 if dt == F16 else cb32)[: sh[0], off : off + cols]
                if len(sh) == 3:
                    v = v.rearrange("p (a b) -> p a b", b=sh[2])
                ct[n] = v

            # persistent state tiles (w = W1 y, f16: it feeds matmuls as rhs)
            w = big.tile([128, HB], F16, tag="w")
            y0 = big.tile([128, HB], F16, tag="y0")
            # one tanh-stage tile per (step, stage): all flow into the regressor
            tS = [[big.tile([128, HB], F16, tag=f"t{s}_{i}", name=f"t{s}_{i}")
                   for i in range(4)] for s in range(steps)]
            pred_sb = big.tile([128, HB // 2], F32, tag="pred")

            for _rep in range(time_reps):

                # ------------- Phase 1: conv + encoder -------------
                def dest_of_group(g):
                    # group g (512 samples) -> (row offset, chunk-col)
                    h_, cc = (0, g) if g < NG // 2 else (1, g - NG // 2)
                    return 64 * h_, cc

                # one shared [128,1024] PSUM pool spans all phases so the
                # scheduler can overlap the encoder tail with the ODE start;
                # eps closes after the encoder and its banks become wp's.
                with tc.tile_pool(name="sb", bufs=3) as epool, \
                     tc.tile_pool(name="ps", bufs=2, space="PSUM") as ps, \
                     tc.tile_pool(name="scr", bufs=4) as opool:
                    AO = mybir.AluOpType
                    with tc.tile_pool(name="ps2", bufs=2, space="PSUM") as eps, \
                         tc.tile_pool(name="cv_ps", bufs=1,
                                      space="PSUM") as cvp:
                        # interleave the two batch halves so each chunk-col's
                        # w0 can be emitted as soon as its pair completes
                        order = [g for p in zip(range(NG // 2),
                                                range(NG // 2, NG))
                                 for g in p]
                        for g in order:
                            ro, cc = dest_of_group(g)
                            ccols = bass.ts(cc, 512)
                            xt = epool.tile([128, NCHUNK, 512], F16, tag="xt")
                            if g == 0:
                                # split so the first conv matmuls start as soon
                                # as the leading chunks land
                                nc.sync.dma_start(out=xt[:, 0:4, :],
                                                  in_=x_in[:, g, 0:4])
                                nc.sync.dma_start(out=xt[:, 4:NCHUNK, :],
                                                  in_=x_in[:, g, 4:NCHUNK])
                            else:
                                nc.sync.dma_start(out=xt[:], in_=x_in[:, g])
                            h_t = epool.tile([128, NCHUNK, 512], F16, tag="h")
                            for b in range(NCHUNK // 2):
                                if b % 3 == 2:
                                    cp = cvp.tile([128, 1024], F32, tag="cvp")
                                else:
                                    cp = ps.tile([128, 1024], F32, tag="ps")
                                for hf in range(2):
                                    j = 2 * b + hf
                                    nc.tensor.matmul(
                                        cp[:, bass.ts(hf, 512)], ct["cv3"][:],
                                        xt[:, j, :], start=True, stop=True,
                                        skip_group_check=True)
                                nc.scalar.activation(
                                    h_t[:, 2 * b : 2 * b + 2, :].rearrange(
                                        "p a b -> p (a b)"),
                                    cp[:], AF.Silu, bias=ct["conv_bias"][:])
                            ep = eps.tile([128, 512], F32, tag="ep")
                            for j in range(NCHUNK):
                                nc.tensor.matmul(ep[:], ct["enc1_w"][:, j, :],
                                                 h_t[:, j, :], start=(j == 0),
                                                 stop=(j == NCHUNK - 1),
                                                 skip_group_check=True)
                            e1 = epool.tile([128, 512], F16, tag="e1")
                            nc.vector.tensor_scalar(
                                out=e1[:], in0=ep[:],
                                scalar1=ct["enc1_bias"][:], scalar2=0.0,
                                op0=AO.add, op1=AO.max)
                            tp = eps.tile([128, 512], F32, tag="ep")
                            nc.tensor.matmul(tp[0:64, :], ct["enc2_w"][:], e1[:],
                                             start=True, stop=True,
                                             skip_group_check=True)
                            nc.vector.tensor_scalar_add(
                                out=y0[ro : ro + 64, ccols], in0=tp[0:64, :],
                                scalar1=ct["enc2_bias"][0:64])
                            if g >= NG // 2:
                                # both halves of chunk-col cc are done:
                                # w0 = W1 @ y0 (block-diagonal over halves)
                                wp0 = eps.tile([128, 512], F32, tag="ep")
                                nc.tensor.matmul(wp0[:], ct["w1t"][:],
                                                 y0[:, ccols], start=True,
                                                 stop=True,
                                                 skip_group_check=True)
                                nc.vector.tensor_copy(out=w[:, ccols],
                                                      in_=wp0[:])

                    # ------------- Phase 2: RK4 ODE -------------
                    def mm2(pst, sl, rhs, vcol, start, stop):
                        """One term: 2 block-diagonal matmuls (512-col chunks)."""
                        lw = ct["osl"]
                        for ch in range(2):
                            cols = bass.ds(1024 * vcol + 512 * ch, 512)
                            nc.tensor.matmul(pst[:, 512 * ch : 512 * ch + 512],
                                             lw[:, sl, :], rhs[:, cols],
                                             start=start, stop=stop,
                                             skip_group_check=True)

                    with tc.tile_pool(name="wp_ps", bufs=2,
                                      space="PSUM") as wpp:
                        wp_prev = [None] * NW
                        for n in range(steps):
                            t = tS[n]
                            for v in range(NW):
                                vc = bass.ts(v, 1024)
                                # t1: step 0 from w (SBUF); later steps read the
                                # previous step's wp PSUM directly (its bias
                                # beta3 = b1 + h*cvec matches w' = wp + gamma)
                                if n == 0:
                                    nc.scalar.activation(
                                        t[0][:, vc], w[:, vc], AF.Tanh,
                                        bias=ct["beta"][:, 0:1])
                                else:
                                    nc.scalar.activation(
                                        t[0][:, vc], wp_prev[v][:], AF.Tanh,
                                        bias=ct["beta"][:, 3:4])
                                # z2, z3, z4: closed PSUM groups (V-term + I w)
                                for i, (sl, tin) in enumerate(
                                        [(0, t[0]), (0, t[1]), (1, t[2])]):
                                    zb = ps.tile([128, 1024], F32, tag="ps")
                                    mm2(zb, 4, w, v, True, False)
                                    mm2(zb, sl, tin, v, False, True)
                                    nc.scalar.activation(
                                        t[i + 1][:, vc], zb[:], AF.Tanh,
                                        bias=ct["beta"][:, i + 1 : i + 2])
                                if n == steps - 1:
                                    continue  # final w never consumed
                                # wp = w + (h/6)V(t1+2t2+2t3+t4)  (closed group)
                                wp = wpp.tile([128, 1024], F32, tag="wp")
                                for k, (sl, tin) in enumerate(
                                        [(4, w), (2, t[0]), (3, t[1]),
                                         (3, t[2]), (2, t[3])]):
                                    mm2(wp, sl, tin, v, k == 0, k == 4)
                                wp_prev[v] = wp
                                # w' = wp + gamma (off the critical path: the
                                # next step's t1 reads wp directly)
                                nc.vector.tensor_scalar_add(
                                    out=w[:, vc], in0=wp[:],
                                    scalar1=ct["gamma"][:])

                        # ------------- Phase 3: regressor -------------
                        # pred = R2 relu(R1 y0 + sum_{s,j} rw_j t_sj + bias_r)
                        for pr in range(NCH // 2):
                            rp = ps.tile([128, 1024], F32, tag="ps")
                            for idx in range(2):
                                cc = 2 * pr + idx
                                ccols = bass.ts(cc, 512)
                                orow = slice(64 * idx, 64 * idx + 64)
                                tp_ = (0, 64 * idx)
                                nc.tensor.matmul(rp[orow, 0:512], ct["r1y"][:],
                                                 y0[:, ccols], start=True,
                                                 stop=False, tile_position=tp_,
                                                 skip_group_check=True)
                                for s in range(steps):
                                    for j in range(4):
                                        nc.tensor.matmul(
                                            rp[orow, 0:512], ct["rw"][:, j, :],
                                            tS[s][j][:, ccols], start=False,
                                            stop=(s == steps - 1 and j == 3),
                                            tile_position=tp_,
                                            skip_group_check=True)
                            rr = opool.tile([128, 512], F16, tag="rr")
                            nc.vector.tensor_scalar(
                                out=rr[:], in0=rp[:, 0:512],
                                scalar1=ct["bias_r"][:], scalar2=0.0,
                                op0=AO.add, op1=AO.max)
                            pp = wpp.tile([128, 1024], F32, tag="wp")
                            nc.tensor.matmul(pp[0:4, 0:512], ct["r2"][:], rr[:],
                                             start=True, stop=True,
                                             skip_group_check=True)
                            nc.vector.tensor_scalar_add(
                                out=pred_sb[0:4, bass.ts(pr, 512)],
                                in0=pp[0:4, 0:512], scalar1=ct["br2"][0:4])

                    # out DMA: pred_sb row k=(h,par) -> sample mapping
                    pv = pred_sb.rearrange("p (q n) -> p q n", n=512)
                    ov = out_t.rearrange("(h q par n) -> h par q n",
                                         h=2, par=2, n=512)
                    npair = NCH // 2
                    for k, (h_, par) in enumerate(
                            [(0, 0), (1, 0), (0, 1), (1, 1)]):
                        nc.sync.dma_start(out=ov[h_, par],
                                          in_=pv[k : k + 1, 0:npair, :])
    nc.compile()
    return nc


_CACHE = {}


def _get_nc(bpc, steps):
    key = (bpc, steps)
    if key not in _CACHE:
        _CACHE[key] = build_nc(bpc, steps)
    return _CACHE[key]


def make_in_maps(inputs):
    x = np.asarray(inputs["x"])
    B = x.shape[0]
    bpc = B // N_CORES
    ng = bpc // 512
    # position-major staging [44, 24, B]: pad positions -1..42 (index p+1)
    xq = np.zeros((SEQ + 4, IN_DIM, B), np.float16)
    xq[1 : SEQ + 1] = np.ascontiguousarray(
        x.astype(np.float16).transpose(1, 2, 0))
    consts = make_consts(inputs)
    b16, b32 = pack_consts(consts)
    base = {"cb16": b16, "cb32": b32}
    in_maps = []
    for c in range(N_CORES):
        xd = np.zeros((128, ng, NCHUNK, 512), np.float16)
        # chunk j rows 24*si+ci = in position 3j-1+si (xq index 3j+si)
        for j in range(NCHUNK):
            for si in range(5):
                src = xq[3 * j + si, :, c * bpc : (c + 1) * bpc]
                xd[24 * si : 24 * si + 24, :, j, :] = src.reshape(
                    IN_DIM, ng, 512)
        in_maps.append(dict(base, xd=xd))
    return bpc, in_maps


def kernel(**inputs):
    bpc, in_maps = make_in_maps(inputs)
    nc = _get_nc(bpc, ODE_STEPS)
    res = run_bass_kernel_spmd(nc, in_maps, list(range(N_CORES)))
    return np.concatenate([res.results[i]["out"] for i in range(N_CORES)])
